# revision 20
# baseline (speedup 1.0000x reference)
"""Character-delimited (segment-local causal) attention on 8 trn2 cores.

Sharding: core = (batch, head-half): b = core//2, hh = core%2.
Each core computes the qkv projection for its batch restricted to its 8
heads (512 of the 3072 Wqkv columns per section) plus the segment-sparse
attention for those heads.

Device pipeline (per core, bf16 matmul operands / fp32 accumulation):
  - x arrives HOST-TRANSPOSED (xT chunk tiles [128, et, 512]) so all x
    loads are plain contiguous DMAs (8KB/partition lines); no xbar
    DMA-transpose and no PE transposes anywhere in the program.
  - w q/k sections are host-packed COLUMN-major ([ot][p][et][c]) so the
    first qk matmul only needs xT chunk 0 plus one 256KB w tile: the PE
    starts ~2.5us into the program and stays dense (HAM stays warm).
  - q,k are produced transposed (qT/kT: [d, s]) so QK^T needs no
    transposes; v is produced in natural [s, d] layout with an extra
    all-ones column per head so the PV matmul also accumulates the
    softmax denominator (row 64 of the ctx psum).
  - Attention is block-sparse: for each 512-query chunk only key blocks
    that can contain same-segment keys are computed (block geometry is
    derived on the host from char_ids and baked into the program; per-
    batch exactness is restored by elementwise 0/1 masks multiplied into
    exp(scores)).  scores are computed transposed ([k, q]) so exp(scores)
    feeds the PV matmul directly as the moving operand.
  - ctx stays TRANSPOSED to the end: ctx^T [65, 512] (64 dims + the
    denominator row) is copied PSUM->SBUF (alternating ACT/DVE) and
    written per head to a transposed DRAM output [520, 2048]; the host
    divides by the denominator row and transposes back while unsharding.
    (A device-side normalize was tried and reverted: a [1,512] DVE
    reciprocal runs single-lane at ~4us, and gpsimd partition_broadcast
    is both slow and wrong on HW.)
  - emission is software-pipelined: qkv for chunk sc interleaves with the
    attention of chunk sc-1, xT/mask loads prefetch one chunk ahead, and
    the last chunk's attention heads are interleaved into its own qkv
    phase so the kernel has no long serial tail.
"""

import numpy as np
import ml_dtypes

B, S, E = 4, 2048, 1024
H, D = 16, 64
NCORES = 8
CH = 512          # query chunk
KB = 128          # key block
NCH = S // CH     # 4 chunks
DELIMS = (32, 10)
HPC = H // 2      # heads per core (8)
ET = E // 128     # 8 e-tiles

_prog_cache = {}


def _segments(char_ids):
    """seg ids, per-position segment start / end (exclusive), per batch."""
    ids = np.asarray(char_ids)
    is_d = np.zeros(ids.shape, dtype=bool)
    for d in DELIMS:
        is_d |= ids == d
    seg = np.cumsum(is_d.astype(np.int64), axis=-1)
    starts = np.empty_like(seg)
    ends = np.empty_like(seg)
    for b in range(seg.shape[0]):
        starts[b] = np.searchsorted(seg[b], seg[b], side="left")
        ends[b] = np.searchsorted(seg[b], seg[b], side="right")
    return seg, starts, ends


def _geometry(seg, starts, ends):
    """Shared (union over batches) block geometry.

    Returns blocks[qc] = list of (k0, qoff, N, moff) and mask width MASKC.
    """
    blocks = []
    maskc = 0
    for qc in range(NCH):
        q0 = qc * CH
        sstart_min = int(starts[:, q0].min())
        send_max = int(ends[:, q0].max())
        past_lo = (sstart_min // KB) * KB
        blist = []
        moff = 0
        for k0 in range(past_lo, q0, KB):
            # queries beyond this block's last key's segment end can never
            # attend into it, so trim the q-extent per block
            qe_blk = min(int(ends[:, k0 + KB - 1].max()) - q0, CH)
            if qe_blk <= 0:
                continue
            blist.append((k0, 0, qe_blk, moff))
            moff += qe_blk
        for kc in range(CH // KB):
            k0 = q0 + kc * KB
            de = int(ends[:, k0 + KB - 1].max())
            de = min(max(de, k0 + KB), q0 + CH)
            n = de - k0
            blist.append((k0, kc * KB, n, moff))
            moff += n
        blocks.append(blist)
        maskc = max(maskc, moff)
    return blocks, maskc


def _masks_for_batch(seg_b, blocks, maskc):
    """[NCH, 128, maskc] bfloat16 0/1 mask blob for one batch."""
    out = np.zeros((NCH, KB, maskc), dtype=ml_dtypes.bfloat16)
    pos = np.arange(S)
    for qc, blist in enumerate(blocks):
        q0 = qc * CH
        for (k0, qoff, n, moff) in blist:
            kk = pos[k0:k0 + KB]
            qq = pos[q0 + qoff:q0 + qoff + n]
            m = (seg_b[kk][:, None] == seg_b[qq][None, :]) & (kk[:, None] <= qq[None, :])
            out[qc, :, moff:moff + n] = m.astype(ml_dtypes.bfloat16)
    return out


def _group_blocks(blist):
    """Pack consecutive blocks into groups whose total q-extent fits one
    512-col psum bank.  Returns [(g_moff, gN, [(k0, qoff, n, moff), ...])]."""
    groups = []
    cur, width = [], 0
    for blk in blist:
        n = blk[2]
        if cur and width + n > CH:
            groups.append((cur[0][3], width, cur))
            cur, width = [], 0
        cur.append(blk)
        width += n
    if cur:
        groups.append((cur[0][3], width, cur))
    return groups


def _build_program(blocks, maskc):
    import concourse.bacc as bacc
    import concourse.tile as tile
    from concourse import mybir
    from contextlib import ExitStack

    f32 = mybir.dt.float32
    bf16 = mybir.dt.bfloat16
    AF = mybir.ActivationFunctionType

    nc = bacc.Bacc("TRN2", target_bir_lowering=False, debug=False,
                   num_devices=NCORES)

    x_h = nc.dram_tensor("x", [NCH, 128, ET * CH], bf16, kind="ExternalInput")
    wqk_h = nc.dram_tensor("wqk", [8, 128, ET * 128], bf16, kind="ExternalInput")
    wv_h = nc.dram_tensor("wv", [128, ET * CH], bf16, kind="ExternalInput")
    bqk_h = nc.dram_tensor("bqk", [128, 8], f32, kind="ExternalInput")
    mk_h = nc.dram_tensor("masks", [NCH, KB, maskc], bf16, kind="ExternalInput")
    out_h = nc.dram_tensor("out", [HPC * 65, S], f32, kind="ExternalOutput")

    with tile.TileContext(nc) as tc:
        with ExitStack() as ctx:
            sing = ctx.enter_context(tc.tile_pool(name="sing", bufs=1))
            xtp = ctx.enter_context(tc.tile_pool(name="xtp", bufs=3))
            qp = ctx.enter_context(tc.tile_pool(name="qp", bufs=3))
            mp = ctx.enter_context(tc.tile_pool(name="mp", bufs=3))
            esp = ctx.enter_context(tc.tile_pool(name="esp", bufs=8))
            ctsp = ctx.enter_context(tc.tile_pool(name="ctsp", bufs=4))

            ph1 = ctx.enter_context(tc.tile_pool(name="ph1", bufs=2, space="PSUM"))
            scrp = ctx.enter_context(tc.tile_pool(name="scrp", bufs=3, space="PSUM"))
            ctxp = ctx.enter_context(tc.tile_pool(name="ctxp", bufs=3, space="PSUM"))

            # ---- persistent tensors (DMA emission order == queue order;
            #      chunk-0 critical path first) ----
            x_tiles = {}
            q_tiles = {}
            mask_tiles = {}

            def x_load(sc):
                xt = xtp.tile([128, ET, CH], bf16, tag="xt", name="xt")
                eng = nc.gpsimd if sc == 0 else nc.sync
                eng.dma_start(out=xt, in_=x_h[sc, :, :])
                x_tiles[sc] = xt
                q_tiles[sc] = qp.tile([128, 4, CH], bf16, tag="q", name="q_t")

            def mask_load(sc):
                mask_t = mp.tile([128, maskc], bf16, tag="m")
                nc.sync.dma_start(out=mask_t, in_=mk_h[sc, :, :])
                mask_tiles[sc] = mask_t

            # PE warm-up spinner: zero matmuls with no DMA deps keep the
            # PE busy from engine-ready (~8us) so HAM flips to K=8/8 right
            # as the first real matmul's weights land (~12us).
            wz_sb = sing.tile([128, CH], bf16, tag="wz")
            nc.vector.memset(wz_sb, 0.0)
            wup = ph1.tile([128, CH], f32, tag="ph1", name="warm")
            for _ in range(7):
                nc.tensor.matmul(wup, wz_sb[:, 0:128], wz_sb,
                                 start=True, stop=True)

            x_load(0)
            wqk_sbs = [sing.tile([128, ET, 128], bf16, tag=f"wqk{ot}",
                                 name=f"wqk{ot}")
                       for ot in range(8)]
            for ot in (0, 4, 1, 5, 2, 6, 3, 7):  # consumption order
                eng = nc.gpsimd if ot == 0 else nc.sync
                eng.dma_start(out=wqk_sbs[ot], in_=wqk_h[ot, :, :])
            bqk_sb = sing.tile([128, 8], f32, tag="bqk")
            nc.sync.dma_start(out=bqk_sb, in_=bqk_h[:, :])
            wv_sb = sing.tile([128, ET, CH], bf16, tag="wv")
            nc.sync.dma_start(out=wv_sb, in_=wv_h[:, :])
            mask_load(0)
            x_load(1)
            mask_load(1)


            k_sbs, v_sbs = [], []
            for c in range(NCH):
                kt_ = sing.tile([128, 4, CH], bf16, tag=f"k{c}")
                vt_ = sing.tile([128, 4, HPC, 65], bf16, tag=f"v{c}")
                nc.vector.memset(vt_[:, :, :, 64:65], 1.0)
                k_sbs.append(kt_)
                v_sbs.append(vt_)

            # -------- phase1 (qkv projection) units --------
            def qk_unit(sc, ot):
                xt = x_tiles[sc]
                pq = ph1.tile([128, CH], f32, tag="ph1")
                for et in range(ET):
                    nc.tensor.matmul(
                        pq, wqk_sbs[ot][:, et, :], xt[:, et, :],
                        start=(et == 0), stop=(et == ET - 1))
                if ot < 4:
                    nc.scalar.add(q_tiles[sc][:, ot, :], pq,
                                  bqk_sb[:, ot:ot + 1])
                else:
                    nc.vector.tensor_scalar_add(k_sbs[sc][:, ot - 4, :],
                                                pq, bqk_sb[:, ot:ot + 1])

            def v_unit(sc, ss):
                xt = x_tiles[sc]
                pv = ph1.tile([128, CH], f32, tag="ph1")
                for et in range(ET):
                    nc.tensor.matmul(
                        pv, xt[:, et, ss * 128:(ss + 1) * 128],
                        wv_sb[:, et, :],
                        start=(et == 0), stop=(et == ET - 1))
                nc.vector.tensor_copy(
                    v_sbs[sc][:, ss, :, 0:64],
                    pv.rearrange("p (h c) -> p h c", c=64))

            def phase1_units(sc, qk_first):
                units = []
                if qk_first:
                    for hp in range(4):
                        units.append(lambda o=hp: qk_unit(sc, o))
                        units.append(lambda o=4 + hp: qk_unit(sc, o))
                    for ss in range(4):
                        units.append(lambda s=ss: v_unit(sc, s))
                else:
                    for ss in range(4):
                        units.append(lambda s=ss: v_unit(sc, s))
                    for hp in range(4):
                        units.append(lambda o=hp: qk_unit(sc, o))
                        units.append(lambda o=4 + hp: qk_unit(sc, o))
                return units

            # -------- attention units (per query chunk) --------
            def make_attn(qc):
                groups = _group_blocks(blocks[qc])
                state = {}

                # PV emission plan: diagonal blocks ascending (split at the
                # previous diagonal's end so each matmul's region is
                # uniformly fresh or uniformly accumulating w.r.t. the PSUM
                # has_written bits), then past blocks (always accumulate,
                # since they lie inside diag-0's fresh region).  The first
                # segment carries start=True, so no zero-init matmul needed.
                flat = []
                for gi, (gm, gn, blks) in enumerate(groups):
                    for (k0, qoff, n, moff) in blks:
                        flat.append((k0, qoff, n, gi, moff - gm))
                q0 = qc * CH
                diag = sorted(f for f in flat if f[0] >= q0)
                past = [f for f in flat if f[0] < q0]
                pv_segs = []
                prev_de = 0
                for (k0, qoff, n, gi, ec) in diag:
                    de = qoff + n
                    cut = min(max(prev_de, qoff), de)
                    if cut > qoff:
                        pv_segs.append((k0, qoff, cut - qoff, gi, ec))
                    if de > cut:
                        pv_segs.append((k0, cut, de - cut, gi,
                                        ec + (cut - qoff)))
                    prev_de = max(prev_de, de)
                pv_segs.extend(past)

                def head_a(h):
                    q_t = q_tiles[qc]
                    mask_t = mask_tiles[qc]
                    p0 = (h % 2) * 64
                    kp = h // 2
                    scrs = []
                    for (gm, gn, blks) in groups:
                        scr = scrp.tile([128, CH], f32, tag="scr")
                        for (k0, qoff, n, moff) in blks:
                            kci, koff = k0 // CH, k0 % CH
                            nc.tensor.matmul(
                                scr[:, moff - gm:moff - gm + n],
                                k_sbs[kci][p0:p0 + 64, kp, koff:koff + 128],
                                q_t[p0:p0 + 64, kp, qoff:qoff + n],
                                start=True, stop=True)
                        scrs.append(scr)
                    ess = []
                    for gi, (gm, gn, blks) in enumerate(groups):
                        es = esp.tile([128, CH], bf16, tag="es")
                        nc.scalar.activation(es[:, 0:gn], scrs[gi][:, 0:gn],
                                             AF.Exp)
                        nc.vector.tensor_mul(es[:, 0:gn], es[:, 0:gn],
                                             mask_t[:, gm:gm + gn])
                        ess.append(es)
                    state[("h", h)] = ess

                def head_b(h):
                    ess = state.pop(("h", h))
                    ctx_t = ctxp.tile([65, CH], f32, tag="ct", name="ctx_t")
                    nseg = len(pv_segs)
                    for i, (k0, s0, slen, gi, ec) in enumerate(pv_segs):
                        kci, koff = k0 // CH, k0 % CH
                        nc.tensor.matmul(
                            ctx_t[:, s0:s0 + slen],
                            v_sbs[kci][:, koff // 128, h, :],
                            ess[gi][:, ec:ec + slen],
                            start=(i == 0), stop=(i == nseg - 1))
                    cts = ctsp.tile([65, CH], f32, tag="cts")
                    if h % 2 == 0:
                        nc.vector.tensor_copy(cts, ctx_t)
                    else:
                        nc.scalar.copy(cts, ctx_t)
                    nc.sync.dma_start(
                        out=out_h[h * 65:(h + 1) * 65, qc * CH:(qc + 1) * CH],
                        in_=cts)

                return head_a, head_b

            def attn_units(qc):
                """1-head-lag ordering: a0 a1 b0 a2 b1 ... a7 b6 b7."""
                head_a, head_b = make_attn(qc)
                order = []
                for h in range(HPC):
                    order.append(lambda h=h: head_a(h))
                    if h >= 1:
                        order.append(lambda h=h - 1: head_b(h))
                order.append(lambda: head_b(HPC - 1))
                return order

            def interleave(a, b):
                """Merge unit lists proportionally (a paced against b)."""
                if not b:
                    return list(a)
                out = []
                na, nb_ = len(a), len(b)
                ia = ib_ = 0
                while ia < na or ib_ < nb_:
                    if ib_ * na <= ia * nb_:
                        if ib_ < nb_:
                            out.append(b[ib_]); ib_ += 1
                        else:
                            out.append(a[ia]); ia += 1
                    else:
                        if ia < na:
                            out.append(a[ia]); ia += 1
                        else:
                            out.append(b[ib_]); ib_ += 1
                return out

            # -------- emission schedule --------
            # chunks 0-2: phase1(sc) interleaved with attn(sc-1), with
            # xT/mask prefetch of sc+1 inserted early in the stream.
            for sc in range(3):
                p1 = phase1_units(sc, qk_first=True)
                att = attn_units(sc - 1) if sc > 0 else []
                stream = interleave(p1, att)
                pf = sc + 2
                if pf <= 3:
                    stream.insert(min(4, len(stream)), lambda c=pf: x_load(c))
                    stream.insert(min(6, len(stream)), lambda c=pf: mask_load(c))
                for u in stream:
                    u()

            # chunk 3: v units first, then qk pairs; attn(2) merged in and
            # attn(3) heads start as soon as their q/k pair lands.
            a2 = attn_units(2)
            ha3, hb3 = make_attn(3)
            v_units3 = [lambda s=ss: v_unit(3, s) for ss in range(4)]
            pre = interleave(v_units3, a2[:6])
            a2_rest = a2[6:]  # 11 units, drained across the qk pairs
            for u in pre:
                u()
            a2_i = 0

            def drain_a2(k):
                nonlocal a2_i
                for _ in range(k):
                    if a2_i < len(a2_rest):
                        a2_rest[a2_i]()
                        a2_i += 1

            for hp in range(4):
                qk_unit(3, hp)
                drain_a2(2)
                qk_unit(3, 4 + hp)
                drain_a2(1)
                if hp > 0:
                    hb3(2 * hp - 2)
                ha3(2 * hp)
                drain_a2(1)
                ha3(2 * hp + 1)
                if hp > 0:
                    hb3(2 * hp - 1)
            drain_a2(len(a2_rest))
            hb3(6)
            hb3(7)
    nc.compile()
    return nc


def _prep_inputs(x, char_ids, Wqkv, bqkv):
    x = np.asarray(x, dtype=np.float32)
    Wqkv = np.asarray(Wqkv, dtype=np.float32)
    bqkv = np.asarray(bqkv, dtype=np.float32)
    seg, starts, ends = _segments(char_ids)
    blocks, maskc = _geometry(seg, starts, ends)
    masks = [_masks_for_batch(seg[b], blocks, maskc) for b in range(B)]

    bf = ml_dtypes.bfloat16
    sq = np.float32(1.0 / np.sqrt(D))
    # host-transposed x, chunked: [sc, p, et, c] = x[sc*512+c, et*128+p]
    xts = []
    for b in range(B):
        xb = x[b].astype(bf)
        xts.append(np.ascontiguousarray(
            xb.reshape(NCH, CH, ET, 128).transpose(0, 3, 2, 1)
        ).reshape(NCH, 128, ET * CH))
    in_maps = []
    bvs = []
    for core in range(NCORES):
        b, hh = core // 2, core % 2
        c0 = hh * CH
        wq = Wqkv[:, c0:c0 + CH] * sq
        wk = Wqkv[:, E + c0:E + c0 + CH]
        wv = Wqkv[:, 2 * E + c0:2 * E + c0 + CH] * np.float32(1.0 / D)
        bq = bqkv[c0:c0 + CH] * sq
        bk = bqkv[E + c0:E + c0 + CH]
        bv = bqkv[2 * E + c0:2 * E + c0 + CH] * np.float32(1.0 / D)
        qk = np.concatenate([wq, wk], axis=1).astype(bf)  # [1024, 1024]
        # [ot, p, et, c] = qk[et*128+p, ot*128+c]
        wqk = np.ascontiguousarray(
            qk.reshape(ET, 128, 8, 128).transpose(2, 1, 0, 3)
        ).reshape(8, 128, ET * 128)
        # [p, et, c] = wv[et*128+p, c]
        wvp = np.ascontiguousarray(
            wv.astype(bf).reshape(ET, 128, CH).transpose(1, 0, 2)
        ).reshape(128, ET * CH)
        bqk = np.ascontiguousarray(
            np.concatenate([bq.reshape(4, 128).T, bk.reshape(4, 128).T], axis=1))
        in_maps.append({
            "x": xts[b],
            "wqk": wqk,
            "wv": wvp,
            "bqk": bqk,
            "masks": masks[b],
        })
        bvs.append(bv.reshape(HPC, 64, 1))
    return in_maps, blocks, maskc, bvs


def _ensure_axon_hook_stub():
    # bass_utils' axon trace path imports antenv.axon_hooks; if the module
    # is absent in this image and BASS_TRACE happens to be set, the import
    # would crash.  Provide a no-op fallback (a real module wins if present).
    try:
        import antenv.axon_hooks  # noqa: F401
    except ImportError:
        import sys
        import types
        mod = types.ModuleType("antenv.axon_hooks")
        mod.get_axon_ntff_profile_hook = lambda: None
        mod.set_axon_ntff_profile_hook = lambda h: None
        sys.modules["antenv.axon_hooks"] = mod


def kernel(x, char_ids, Wqkv, bqkv):
    from concourse.bass_utils import run_bass_kernel_spmd

    _ensure_axon_hook_stub()

    in_maps, blocks, maskc, bvs = _prep_inputs(x, char_ids, Wqkv, bqkv)
    key = repr((tuple(tuple(b) for b in blocks), maskc))
    if key not in _prog_cache:
        _prog_cache[key] = _build_program(blocks, maskc)
    nc = _prog_cache[key]

    out = np.empty((B, S, E), dtype=np.float32)
    for attempt in range(3):
        res = run_bass_kernel_spmd(nc, in_maps, list(range(NCORES)))
        for core in range(NCORES):
            b, hh = core // 2, core % 2
            r = res.results[core]["out"].reshape(HPC, 65, S)
            ctx = r[:, 0:64, :] / r[:, 64:65, :] + bvs[core]
            out[b, :, hh * CH:(hh + 1) * CH] = (
                ctx.reshape(HPC * 64, S).T)
        if np.isfinite(out).all():
            break
    return out


# revision 21
# speedup vs baseline: 1.0071x; 1.0071x over previous
"""Character-delimited (segment-local causal) attention on 8 trn2 cores.

Sharding: core = (batch, head-half): b = core//2, hh = core%2.
Each core computes the qkv projection for its batch restricted to its 8
heads (512 of the 3072 Wqkv columns per section) plus the segment-sparse
attention for those heads.

Device pipeline (per core, bf16 matmul operands / fp32 accumulation):
  - x arrives HOST-TRANSPOSED (xT chunk tiles [128, et, 512]) so all x
    loads are plain contiguous DMAs (8KB/partition lines); no xbar
    DMA-transpose and no PE transposes anywhere in the program.
  - w q/k sections are host-packed COLUMN-major ([ot][p][et][c]) so the
    first qk matmul only needs xT chunk 0 plus one 256KB w tile: the PE
    starts ~2.5us into the program and stays dense (HAM stays warm).
  - q,k are produced transposed (qT/kT: [d, s]) so QK^T needs no
    transposes; v is produced in natural [s, d] layout with an extra
    all-ones column per head so the PV matmul also accumulates the
    softmax denominator (row 64 of the ctx psum).
  - Attention is block-sparse: for each 512-query chunk only key blocks
    that can contain same-segment keys are computed (block geometry is
    derived on the host from char_ids and baked into the program; per-
    batch exactness is restored by elementwise 0/1 masks multiplied into
    exp(scores)).  scores are computed transposed ([k, q]) so exp(scores)
    feeds the PV matmul directly as the moving operand.
  - ctx stays TRANSPOSED to the end: ctx^T [65, 512] (64 dims + the
    denominator row) is copied PSUM->SBUF (alternating ACT/DVE) and
    written per head to a transposed DRAM output [520, 2048]; the host
    divides by the denominator row and transposes back while unsharding.
    (A device-side normalize was tried and reverted: a [1,512] DVE
    reciprocal runs single-lane at ~4us, and gpsimd partition_broadcast
    is both slow and wrong on HW.)
  - emission is software-pipelined: qkv for chunk sc interleaves with the
    attention of chunk sc-1, xT/mask loads prefetch one chunk ahead, and
    the last chunk's attention heads are interleaved into its own qkv
    phase so the kernel has no long serial tail.
"""

import numpy as np
import ml_dtypes

B, S, E = 4, 2048, 1024
H, D = 16, 64
NCORES = 8
CH = 512          # query chunk
KB = 128          # key block
NCH = S // CH     # 4 chunks
DELIMS = (32, 10)
HPC = H // 2      # heads per core (8)
ET = E // 128     # 8 e-tiles

_prog_cache = {}


def _segments(char_ids):
    """seg ids, per-position segment start / end (exclusive), per batch."""
    ids = np.asarray(char_ids)
    is_d = np.zeros(ids.shape, dtype=bool)
    for d in DELIMS:
        is_d |= ids == d
    seg = np.cumsum(is_d.astype(np.int64), axis=-1)
    starts = np.empty_like(seg)
    ends = np.empty_like(seg)
    for b in range(seg.shape[0]):
        starts[b] = np.searchsorted(seg[b], seg[b], side="left")
        ends[b] = np.searchsorted(seg[b], seg[b], side="right")
    return seg, starts, ends


def _geometry(seg, starts, ends):
    """Shared (union over batches) block geometry.

    Returns blocks[qc] = list of (k0, qoff, N, moff) and mask width MASKC.
    """
    blocks = []
    maskc = 0
    for qc in range(NCH):
        q0 = qc * CH
        sstart_min = int(starts[:, q0].min())
        send_max = int(ends[:, q0].max())
        past_lo = (sstart_min // KB) * KB
        blist = []
        moff = 0
        for k0 in range(past_lo, q0, KB):
            # queries beyond this block's last key's segment end can never
            # attend into it, so trim the q-extent per block
            qe_blk = min(int(ends[:, k0 + KB - 1].max()) - q0, CH)
            if qe_blk <= 0:
                continue
            blist.append((k0, 0, qe_blk, moff))
            moff += qe_blk
        for kc in range(CH // KB):
            k0 = q0 + kc * KB
            de = int(ends[:, k0 + KB - 1].max())
            de = min(max(de, k0 + KB), q0 + CH)
            n = de - k0
            blist.append((k0, kc * KB, n, moff))
            moff += n
        blocks.append(blist)
        maskc = max(maskc, moff)
    return blocks, maskc


def _masks_for_batch(seg_b, blocks, maskc):
    """[NCH, 128, maskc] bfloat16 0/1 mask blob for one batch."""
    out = np.zeros((NCH, KB, maskc), dtype=ml_dtypes.bfloat16)
    pos = np.arange(S)
    for qc, blist in enumerate(blocks):
        q0 = qc * CH
        for (k0, qoff, n, moff) in blist:
            kk = pos[k0:k0 + KB]
            qq = pos[q0 + qoff:q0 + qoff + n]
            m = (seg_b[kk][:, None] == seg_b[qq][None, :]) & (kk[:, None] <= qq[None, :])
            out[qc, :, moff:moff + n] = m.astype(ml_dtypes.bfloat16)
    return out


def _group_blocks(blist):
    """Pack consecutive blocks into groups whose total q-extent fits one
    512-col psum bank.  Returns [(g_moff, gN, [(k0, qoff, n, moff), ...])]."""
    groups = []
    cur, width = [], 0
    for blk in blist:
        n = blk[2]
        if cur and width + n > CH:
            groups.append((cur[0][3], width, cur))
            cur, width = [], 0
        cur.append(blk)
        width += n
    if cur:
        groups.append((cur[0][3], width, cur))
    return groups


def _build_program(blocks, maskc):
    import concourse.bacc as bacc
    import concourse.tile as tile
    from concourse import mybir
    from contextlib import ExitStack

    f32 = mybir.dt.float32
    bf16 = mybir.dt.bfloat16
    AF = mybir.ActivationFunctionType

    nc = bacc.Bacc("TRN2", target_bir_lowering=False, debug=False,
                   num_devices=NCORES)

    x_h = nc.dram_tensor("x", [NCH, 128, ET * CH], bf16, kind="ExternalInput")
    wqk_h = nc.dram_tensor("wqk", [8, 128, ET * 128], bf16, kind="ExternalInput")
    wv_h = nc.dram_tensor("wv", [128, ET * CH], bf16, kind="ExternalInput")
    bqk_h = nc.dram_tensor("bqk", [128, 8], f32, kind="ExternalInput")
    mk_h = nc.dram_tensor("masks", [NCH, KB, maskc], bf16, kind="ExternalInput")
    out_h = nc.dram_tensor("out", [HPC * 65, S], f32, kind="ExternalOutput")

    with tile.TileContext(nc) as tc:
        with ExitStack() as ctx:
            sing = ctx.enter_context(tc.tile_pool(name="sing", bufs=1))
            xtp = ctx.enter_context(tc.tile_pool(name="xtp", bufs=3))
            qp = ctx.enter_context(tc.tile_pool(name="qp", bufs=3))
            mp = ctx.enter_context(tc.tile_pool(name="mp", bufs=3))
            esp = ctx.enter_context(tc.tile_pool(name="esp", bufs=8))
            ctsp = ctx.enter_context(tc.tile_pool(name="ctsp", bufs=4))

            ph1 = ctx.enter_context(tc.tile_pool(name="ph1", bufs=2, space="PSUM"))
            scrp = ctx.enter_context(tc.tile_pool(name="scrp", bufs=3, space="PSUM"))
            ctxp = ctx.enter_context(tc.tile_pool(name="ctxp", bufs=3, space="PSUM"))

            # ---- persistent tensors (DMA emission order == queue order;
            #      chunk-0 critical path first) ----
            x_tiles = {}
            q_tiles = {}
            mask_tiles = {}

            def x_load(sc):
                xt = xtp.tile([128, ET, CH], bf16, tag="xt", name="xt")
                nc.sync.dma_start(out=xt, in_=x_h[sc, :, :])
                x_tiles[sc] = xt
                q_tiles[sc] = qp.tile([128, 4, CH], bf16, tag="q", name="q_t")

            def mask_load(sc):
                mask_t = mp.tile([128, maskc], bf16, tag="m")
                nc.sync.dma_start(out=mask_t, in_=mk_h[sc, :, :])
                mask_tiles[sc] = mask_t

            # PE warm-up spinner: zero matmuls with no DMA deps keep the
            # PE busy from engine-ready (~8us) until the first DMA completion
            # semaphores become visible (~13-15us; ring startup + completion
            # latency dominate, not transfer time), and flip HAM to K=8/8.
            # 16 spins measured optimal; shorter spinners, SWDGE first-loads,
            # and scalar-ring loads all regress 2-3us.
            wz_sb = sing.tile([128, CH], bf16, tag="wz")
            nc.vector.memset(wz_sb, 0.0)
            wup = ph1.tile([128, CH], f32, tag="ph1", name="warm")
            for _ in range(16):
                nc.tensor.matmul(wup, wz_sb[:, 0:128], wz_sb,
                                 start=True, stop=True)

            x_load(0)
            wqk_sbs = [sing.tile([128, ET, 128], bf16, tag=f"wqk{ot}",
                                 name=f"wqk{ot}")
                       for ot in range(8)]
            for ot in (0, 4, 1, 5, 2, 6, 3, 7):  # consumption order
                nc.sync.dma_start(out=wqk_sbs[ot], in_=wqk_h[ot, :, :])
            bqk_sb = sing.tile([128, 8], f32, tag="bqk")
            nc.sync.dma_start(out=bqk_sb, in_=bqk_h[:, :])
            wv_sb = sing.tile([128, ET, CH], bf16, tag="wv")
            nc.sync.dma_start(out=wv_sb, in_=wv_h[:, :])
            mask_load(0)
            x_load(1)
            mask_load(1)


            k_sbs, v_sbs = [], []
            for c in range(NCH):
                kt_ = sing.tile([128, 4, CH], bf16, tag=f"k{c}")
                vt_ = sing.tile([128, 4, HPC, 65], bf16, tag=f"v{c}")
                nc.vector.memset(vt_[:, :, :, 64:65], 1.0)
                k_sbs.append(kt_)
                v_sbs.append(vt_)

            # -------- phase1 (qkv projection) units --------
            def qk_unit(sc, ot):
                xt = x_tiles[sc]
                pq = ph1.tile([128, CH], f32, tag="ph1")
                for et in range(ET):
                    nc.tensor.matmul(
                        pq, wqk_sbs[ot][:, et, :], xt[:, et, :],
                        start=(et == 0), stop=(et == ET - 1))
                if ot < 4:
                    nc.scalar.add(q_tiles[sc][:, ot, :], pq,
                                  bqk_sb[:, ot:ot + 1])
                else:
                    nc.vector.tensor_scalar_add(k_sbs[sc][:, ot - 4, :],
                                                pq, bqk_sb[:, ot:ot + 1])

            def v_unit(sc, ss):
                xt = x_tiles[sc]
                pv = ph1.tile([128, CH], f32, tag="ph1")
                for et in range(ET):
                    nc.tensor.matmul(
                        pv, xt[:, et, ss * 128:(ss + 1) * 128],
                        wv_sb[:, et, :],
                        start=(et == 0), stop=(et == ET - 1))
                nc.vector.tensor_copy(
                    v_sbs[sc][:, ss, :, 0:64],
                    pv.rearrange("p (h c) -> p h c", c=64))

            def phase1_units(sc, qk_first):
                units = []
                if qk_first:
                    for hp in range(4):
                        units.append(lambda o=hp: qk_unit(sc, o))
                        units.append(lambda o=4 + hp: qk_unit(sc, o))
                    for ss in range(4):
                        units.append(lambda s=ss: v_unit(sc, s))
                else:
                    for ss in range(4):
                        units.append(lambda s=ss: v_unit(sc, s))
                    for hp in range(4):
                        units.append(lambda o=hp: qk_unit(sc, o))
                        units.append(lambda o=4 + hp: qk_unit(sc, o))
                return units

            # -------- attention units (per query chunk) --------
            def make_attn(qc):
                groups = _group_blocks(blocks[qc])
                state = {}

                # PV emission plan: diagonal blocks ascending (split at the
                # previous diagonal's end so each matmul's region is
                # uniformly fresh or uniformly accumulating w.r.t. the PSUM
                # has_written bits), then past blocks (always accumulate,
                # since they lie inside diag-0's fresh region).  The first
                # segment carries start=True, so no zero-init matmul needed.
                flat = []
                for gi, (gm, gn, blks) in enumerate(groups):
                    for (k0, qoff, n, moff) in blks:
                        flat.append((k0, qoff, n, gi, moff - gm))
                q0 = qc * CH
                diag = sorted(f for f in flat if f[0] >= q0)
                past = [f for f in flat if f[0] < q0]
                pv_segs = []
                prev_de = 0
                for (k0, qoff, n, gi, ec) in diag:
                    de = qoff + n
                    cut = min(max(prev_de, qoff), de)
                    if cut > qoff:
                        pv_segs.append((k0, qoff, cut - qoff, gi, ec))
                    if de > cut:
                        pv_segs.append((k0, cut, de - cut, gi,
                                        ec + (cut - qoff)))
                    prev_de = max(prev_de, de)
                pv_segs.extend(past)

                def head_a(h):
                    q_t = q_tiles[qc]
                    mask_t = mask_tiles[qc]
                    p0 = (h % 2) * 64
                    kp = h // 2
                    scrs = []
                    for (gm, gn, blks) in groups:
                        scr = scrp.tile([128, CH], f32, tag="scr")
                        for (k0, qoff, n, moff) in blks:
                            kci, koff = k0 // CH, k0 % CH
                            nc.tensor.matmul(
                                scr[:, moff - gm:moff - gm + n],
                                k_sbs[kci][p0:p0 + 64, kp, koff:koff + 128],
                                q_t[p0:p0 + 64, kp, qoff:qoff + n],
                                start=True, stop=True)
                        scrs.append(scr)
                    ess = []
                    for gi, (gm, gn, blks) in enumerate(groups):
                        es = esp.tile([128, CH], bf16, tag="es")
                        nc.scalar.activation(es[:, 0:gn], scrs[gi][:, 0:gn],
                                             AF.Exp)
                        nc.vector.tensor_mul(es[:, 0:gn], es[:, 0:gn],
                                             mask_t[:, gm:gm + gn])
                        ess.append(es)
                    state[("h", h)] = ess

                def head_b(h):
                    ess = state.pop(("h", h))
                    ctx_t = ctxp.tile([65, CH], f32, tag="ct", name="ctx_t")
                    nseg = len(pv_segs)
                    for i, (k0, s0, slen, gi, ec) in enumerate(pv_segs):
                        kci, koff = k0 // CH, k0 % CH
                        nc.tensor.matmul(
                            ctx_t[:, s0:s0 + slen],
                            v_sbs[kci][:, koff // 128, h, :],
                            ess[gi][:, ec:ec + slen],
                            start=(i == 0), stop=(i == nseg - 1))
                    cts = ctsp.tile([65, CH], f32, tag="cts")
                    if h % 2 == 0:
                        nc.vector.tensor_copy(cts, ctx_t)
                    else:
                        nc.scalar.copy(cts, ctx_t)
                    nc.sync.dma_start(
                        out=out_h[h * 65:(h + 1) * 65, qc * CH:(qc + 1) * CH],
                        in_=cts)

                return head_a, head_b

            def attn_units(qc):
                """1-head-lag ordering: a0 a1 b0 a2 b1 ... a7 b6 b7."""
                head_a, head_b = make_attn(qc)
                order = []
                for h in range(HPC):
                    order.append(lambda h=h: head_a(h))
                    if h >= 1:
                        order.append(lambda h=h - 1: head_b(h))
                order.append(lambda: head_b(HPC - 1))
                return order

            def interleave(a, b):
                """Merge unit lists proportionally (a paced against b)."""
                if not b:
                    return list(a)
                out = []
                na, nb_ = len(a), len(b)
                ia = ib_ = 0
                while ia < na or ib_ < nb_:
                    if ib_ * na <= ia * nb_:
                        if ib_ < nb_:
                            out.append(b[ib_]); ib_ += 1
                        else:
                            out.append(a[ia]); ia += 1
                    else:
                        if ia < na:
                            out.append(a[ia]); ia += 1
                        else:
                            out.append(b[ib_]); ib_ += 1
                return out

            # -------- emission schedule --------
            # chunks 0-2: phase1(sc) interleaved with attn(sc-1), with
            # xT/mask prefetch of sc+1 inserted early in the stream.
            for sc in range(3):
                p1 = phase1_units(sc, qk_first=True)
                att = attn_units(sc - 1) if sc > 0 else []
                stream = interleave(p1, att)
                pf = sc + 2
                if pf <= 3:
                    stream.insert(min(4, len(stream)), lambda c=pf: x_load(c))
                    stream.insert(min(6, len(stream)), lambda c=pf: mask_load(c))
                for u in stream:
                    u()

            # chunk 3: v units first, then qk pairs; attn(2) merged in and
            # attn(3) heads start as soon as their q/k pair lands.
            a2 = attn_units(2)
            ha3, hb3 = make_attn(3)
            v_units3 = [lambda s=ss: v_unit(3, s) for ss in range(4)]
            pre = interleave(v_units3, a2[:6])
            a2_rest = a2[6:]  # 11 units, drained across the qk pairs
            for u in pre:
                u()
            a2_i = 0

            def drain_a2(k):
                nonlocal a2_i
                for _ in range(k):
                    if a2_i < len(a2_rest):
                        a2_rest[a2_i]()
                        a2_i += 1

            for hp in range(4):
                qk_unit(3, hp)
                drain_a2(2)
                qk_unit(3, 4 + hp)
                drain_a2(1)
                if hp > 0:
                    hb3(2 * hp - 2)
                ha3(2 * hp)
                drain_a2(1)
                ha3(2 * hp + 1)
                if hp > 0:
                    hb3(2 * hp - 1)
            drain_a2(len(a2_rest))
            hb3(6)
            hb3(7)
    nc.compile()
    return nc


def _prep_inputs(x, char_ids, Wqkv, bqkv):
    x = np.asarray(x, dtype=np.float32)
    Wqkv = np.asarray(Wqkv, dtype=np.float32)
    bqkv = np.asarray(bqkv, dtype=np.float32)
    seg, starts, ends = _segments(char_ids)
    blocks, maskc = _geometry(seg, starts, ends)
    masks = [_masks_for_batch(seg[b], blocks, maskc) for b in range(B)]

    bf = ml_dtypes.bfloat16
    sq = np.float32(1.0 / np.sqrt(D))
    # host-transposed x, chunked: [sc, p, et, c] = x[sc*512+c, et*128+p]
    xts = []
    for b in range(B):
        xb = x[b].astype(bf)
        xts.append(np.ascontiguousarray(
            xb.reshape(NCH, CH, ET, 128).transpose(0, 3, 2, 1)
        ).reshape(NCH, 128, ET * CH))
    in_maps = []
    bvs = []
    for core in range(NCORES):
        b, hh = core // 2, core % 2
        c0 = hh * CH
        wq = Wqkv[:, c0:c0 + CH] * sq
        wk = Wqkv[:, E + c0:E + c0 + CH]
        wv = Wqkv[:, 2 * E + c0:2 * E + c0 + CH] * np.float32(1.0 / D)
        bq = bqkv[c0:c0 + CH] * sq
        bk = bqkv[E + c0:E + c0 + CH]
        bv = bqkv[2 * E + c0:2 * E + c0 + CH] * np.float32(1.0 / D)
        qk = np.concatenate([wq, wk], axis=1).astype(bf)  # [1024, 1024]
        # [ot, p, et, c] = qk[et*128+p, ot*128+c]
        wqk = np.ascontiguousarray(
            qk.reshape(ET, 128, 8, 128).transpose(2, 1, 0, 3)
        ).reshape(8, 128, ET * 128)
        # [p, et, c] = wv[et*128+p, c]
        wvp = np.ascontiguousarray(
            wv.astype(bf).reshape(ET, 128, CH).transpose(1, 0, 2)
        ).reshape(128, ET * CH)
        bqk = np.ascontiguousarray(
            np.concatenate([bq.reshape(4, 128).T, bk.reshape(4, 128).T], axis=1))
        in_maps.append({
            "x": xts[b],
            "wqk": wqk,
            "wv": wvp,
            "bqk": bqk,
            "masks": masks[b],
        })
        bvs.append(bv.reshape(HPC, 64, 1))
    return in_maps, blocks, maskc, bvs


def _ensure_axon_hook_stub():
    # bass_utils' axon trace path imports antenv.axon_hooks; if the module
    # is absent in this image and BASS_TRACE happens to be set, the import
    # would crash.  Provide a no-op fallback (a real module wins if present).
    try:
        import antenv.axon_hooks  # noqa: F401
    except ImportError:
        import sys
        import types
        mod = types.ModuleType("antenv.axon_hooks")
        mod.get_axon_ntff_profile_hook = lambda: None
        mod.set_axon_ntff_profile_hook = lambda h: None
        sys.modules["antenv.axon_hooks"] = mod


def kernel(x, char_ids, Wqkv, bqkv):
    from concourse.bass_utils import run_bass_kernel_spmd

    _ensure_axon_hook_stub()

    in_maps, blocks, maskc, bvs = _prep_inputs(x, char_ids, Wqkv, bqkv)
    key = repr((tuple(tuple(b) for b in blocks), maskc))
    if key not in _prog_cache:
        _prog_cache[key] = _build_program(blocks, maskc)
    nc = _prog_cache[key]

    out = np.empty((B, S, E), dtype=np.float32)
    for attempt in range(3):
        res = run_bass_kernel_spmd(nc, in_maps, list(range(NCORES)))
        for core in range(NCORES):
            b, hh = core // 2, core % 2
            r = res.results[core]["out"].reshape(HPC, 65, S)
            ctx = r[:, 0:64, :] / r[:, 64:65, :] + bvs[core]
            out[b, :, hh * CH:(hh + 1) * CH] = (
                ctx.reshape(HPC * 64, S).T)
        if np.isfinite(out).all():
            break
    return out
